# revision 11
# baseline (speedup 1.0000x reference)
"""Trainium2 Bass kernel for nn_CrossModalAttentionFusion.

Math: the module's two MultiheadAttention blocks run with sequence length 1,
so the softmax over a length-1 axis is identically 1.0 and q/k never affect
the output: each MHA reduces to  out = (fused @ Wv.T + bv) @ Wo.T + bo.
Folding the affine chains (done host-side, on the ~1M-param weights only):

    S        = text + image                      (host, elementwise)
    fused    = S @ fuse_w.T + fuse_b
    out_text = S @ (Wt @ fuse_w).T + (Wt @ fuse_b + t_out_w @ t_bv + t_out_b)
               where Wt = t_out_w @ t_wv
    out_image analogously.

Device work: one [B,1024] @ [1024,3072] matmul (the three projections
concatenated along the output dim), sharded batch-parallel over 8 cores.
"""

import numpy as np

import concourse.bass as bass  # noqa: F401  (registers engine methods)
import concourse.mybir as mybir
import concourse.tile as tile
from concourse import bacc
from concourse.bass_utils import run_bass_kernel_spmd

D = 1024
B = 16384
NCORES = 8
BS = B // NCORES          # 2048 rows per core
PT = 128                  # partition tile
MT = BS // PT             # 16 m-tiles per core
KT = D // PT              # 8 k-tiles (contraction)
NOUT = 3 * D              # fused | out_text | out_image
NFREE = 512               # moving free dim per matmul (one PSUM bank of f32)
NT = NOUT // NFREE        # 6

MM_DT = mybir.dt.float16   # fp16 in / fp32 psum accumulate
NP_DT = np.float16
OUT_DT = mybir.dt.float16  # fp16 out (host upcasts); halves output DMA
ORDER = "kn"               # k outer: stationary S-tile serves 6 matmuls (dedup)

_COMPILED = None
LAST_RESULTS = None       # BassKernelResults from the most recent run


def _dedup_ldweights(nc):
    """Remove InstLdweights that reload the stationary already in the PE.

    The bass add_instruction path emits an LDWEIGHTS before every MATMUL.
    With a k-outer/n-inner loop the same stationary serves 6 consecutive
    matmuls, so 5 of 6 loads are redundant; on HW each costs ~50ns of PE
    time (serial with the matmul stream, no pull-ahead for full-array
    loads). Only removes an LDW when (a) its weights AP is byte-identical
    to the previous LDW on the same block with no other LDW between, and
    (b) it carries no semaphore waits/updates (Tile's buffer-reuse gating
    rides on the matmuls' own updates, which are untouched).
    """
    removed = 0
    for f in nc.m.functions:
        for b in f.blocks:
            last_sig = None
            to_remove = []
            for ins in b.instructions:
                tn = type(ins).__name__
                if tn == "InstLdweights":
                    ap = ins.ins[0]
                    sig = (ap.memref, ap.offset, str(ap.ap), str(ap.dtype),
                           getattr(ins, "perf_mode", None),
                           getattr(ins, "is_transpose", None))
                    si = ins.sync_info
                    clean = si is None or (not si.on_wait and not si.on_update)
                    if sig == last_sig and clean:
                        to_remove.append(ins)
                    else:
                        last_sig = sig
                elif tn == "InstMatmult":
                    pass  # keeps the loaded stationary
            for ins in to_remove:
                b.instructions.remove(ins)
                removed += 1
    return removed


def _build(repeat=1, timing=False, order=ORDER, mm_dt=None, micro=None,
           nfree=NFREE, mtiles=MT, skip_w=False, tweak=True, dedup=1):
    """Build the per-core NEFF.

    timing=True swaps the big external tensors for Internal DRAM (no host
    transfer) and adds a `repeat` loop over the whole batch so device time
    can be extracted by wall-clock differencing between two repeat counts.

    order: "nk" = n outer / k inner (sequential psum groups);
           "kn" = k outer / n inner (stationary reused for NT consecutive mms).
    """
    MM_DT = mm_dt if mm_dt is not None else globals()["MM_DT"]
    NT = NOUT // nfree
    psum_bufs = min(8, (8 * 512) // nfree)
    nc = bacc.Bacc("TRN2", target_bir_lowering=False, debug=False,
                   num_devices=NCORES)
    if timing:
        st = nc.dram_tensor("st", [KT, PT, PT], MM_DT, kind="ExternalInput")
        w = nc.dram_tensor("w", [KT, PT, NOUT], MM_DT)
        bi = nc.dram_tensor("bi", [PT, NOUT], MM_DT)
        out = nc.dram_tensor("out", [BS, NOUT], OUT_DT)
        tok = nc.dram_tensor("tok", [1, 1], mybir.dt.float32,
                             kind="ExternalOutput")
    else:
        st = nc.dram_tensor("st", [MT, KT, PT, PT], MM_DT, kind="ExternalInput")
        w = nc.dram_tensor("w", [KT, PT, NOUT], MM_DT, kind="ExternalInput")
        bi = nc.dram_tensor("bi", [PT, NOUT], MM_DT, kind="ExternalInput")
        out = nc.dram_tensor("out", [BS, NOUT], OUT_DT,
                             kind="ExternalOutput")
        tok = None

    with tile.TileContext(nc) as tc:
        with (
            tc.tile_pool(name="wpool", bufs=1) as wpool,
            tc.tile_pool(name="spool", bufs=32 if tweak == 2 else 24) as spool,
            tc.tile_pool(name="opool", bufs=4 if tweak else 3) as opool,
            tc.tile_pool(name="ppool", bufs=psum_bufs, space="PSUM") as ppool,
        ):
            # W/bias (one-time) and output stores ride the ACT HWDGE queue;
            # S loads get the SP queue to themselves so the PE's critical
            # path never waits behind bulk traffic.
            wts = []
            wt_pending = []
            bt = None
            if not skip_w:
                # Even-k W tiles stream on the ACT queue immediately; odd-k
                # tiles ride the SP queue but must be issued AFTER m0's S
                # tiles (deferred into body) so the PE's first stationaries
                # aren't queued behind bulk weights. Interleaved arrival
                # keeps every wt_k ahead of the PE's k-outer consumption.
                for k in range(KT):
                    wt = wpool.tile([PT, NOUT], MM_DT, name=f"wt{k}",
                                    tag=f"wt{k}")
                    if k % 2 == 0:
                        nc.scalar.dma_start(wt[:], w[k])
                    else:
                        wt_pending.append((wt, k))
                    wts.append(wt)
                bt = wpool.tile([PT, NOUT], MM_DT, name="bt",
                                tag="bt")

            fixed_sts = None
            if micro in ("mm", "mmdve"):
                fixed_sts = []
                for k in range(KT):
                    s_t = wpool.tile([PT, PT], MM_DT, name=f"sf{k}",
                                     tag=f"sf{k}")
                    nc.sync.dma_start(s_t[:], st[k] if timing else st[0, k])
                    fixed_sts.append(s_t)

            bias_loaded = [False]

            def body():
                for m in range(mtiles):
                    if fixed_sts is not None:
                        sts = fixed_sts
                    else:
                        sts = []
                        for k in range(KT):
                            s_t = spool.tile([PT, PT], MM_DT,
                                             name=f"s{m}_{k}", tag="s")
                            nc.sync.dma_start(s_t[:],
                                              st[k] if timing else st[m, k])
                            sts.append(s_t)
                    if bt is not None and not bias_loaded[0]:
                        # SP queue order after m0's S tiles: odd-k W tiles
                        # first (needed by ~4-14us), then bias (first drain
                        # needs it at ~15us).
                        for wt, k in wt_pending:
                            nc.sync.dma_start(wt[:], w[k])
                        nc.sync.dma_start(bt[:], bi[:])
                        bias_loaded[0] = True
                    ot = opool.tile([PT, NOUT], OUT_DT,
                                    name=f"o{m}", tag="o")
                    pts = [ppool.tile([PT, nfree], mybir.dt.float32,
                                      name=f"p{m}_{n}", tag="p")
                           for n in range(NT)]
                    m_order = order
                    if order == "auto":
                        m_order = "kn" if m == 0 else "nk"
                    if m_order == "nk":
                        for n in range(NT):
                            nsl = slice(n * nfree, (n + 1) * nfree)
                            for k in range(KT):
                                nc.tensor.matmul(
                                    pts[n][:], sts[k][:], wts[k][:, nsl],
                                    start=(k == 0), stop=(k == KT - 1),
                                )
                            if tweak and micro != "mm":
                                # drain each group as soon as it closes
                                nc.vector.tensor_add(ot[:, nsl], pts[n][:],
                                                     bt[:, nsl])
                            if tweak and micro is None:
                                if tweak == 2:
                                    chunks = {1: slice(0, 1024),
                                              3: slice(1024, 2048),
                                              5: slice(2048, 3072)}
                                else:
                                    chunks = {NT // 2 - 1: slice(0, NOUT // 2),
                                              NT - 1: slice(NOUT // 2, NOUT)}
                                if n in chunks:
                                    half = chunks[n]
                                    nc.scalar.dma_start(
                                        out[m * PT:(m + 1) * PT, half],
                                        ot[:, half])
                    else:  # "kn"
                        for k in range(KT):
                            for n in range(NT):
                                nsl = slice(n * nfree, (n + 1) * nfree)
                                nc.tensor.matmul(
                                    pts[n][:], sts[k][:], wts[k][:, nsl],
                                    start=(k == 0), stop=(k == KT - 1),
                                )
                    drained = tweak and m_order == "nk"
                    if micro != "mm" and not drained:
                        for n in range(NT):
                            nsl = slice(n * nfree, (n + 1) * nfree)
                            nc.vector.tensor_add(ot[:, nsl], pts[n][:],
                                                 bt[:, nsl])
                    if micro is None and not drained:
                        nc.scalar.dma_start(out[m * PT:(m + 1) * PT, :], ot[:])

            if repeat > 1:
                with tc.For_i(0, repeat, 1,
                              hint_engines=(mybir.EngineType.PE,)):
                    body()
            else:
                body()
            if tok is not None:
                tk = wpool.tile([1, 1], mybir.dt.float32, name="tk", tag="tk")
                nc.gpsimd.memset(tk[:], 1.0)
                nc.sync.dma_start(tok[:], tk[:])

    if dedup:
        _dedup_ldweights(nc)
    nc.compile()
    return nc


def _fold_params(fuse_w, fuse_b, t_in_w, t_in_b, t_out_w, t_out_b,
                 i_in_w, i_in_b, i_out_w, i_out_b):
    """Host-side weight folding (float64). Returns W_all [D, 3D], bias_all [3D]."""
    f8 = np.float64
    fuse_w8, fuse_b8 = fuse_w.astype(f8), fuse_b.astype(f8)

    def fold(in_w, in_b, out_w, out_b):
        wv = in_w[2 * D:3 * D].astype(f8)
        bv = in_b[2 * D:3 * D].astype(f8)
        Wm = out_w.astype(f8) @ wv                    # fused -> out
        bm = out_w.astype(f8) @ bv + out_b.astype(f8)
        W2 = Wm @ fuse_w8                             # S -> out
        b2 = Wm @ fuse_b8 + bm
        return W2, b2

    Wt2, bias_t = fold(t_in_w, t_in_b, t_out_w, t_out_b)
    Wi2, bias_i = fold(i_in_w, i_in_b, i_out_w, i_out_b)

    W_all = np.empty((D, NOUT), np.float32)
    W_all[:, 0:D] = fuse_w8.T
    W_all[:, D:2 * D] = Wt2.T
    W_all[:, 2 * D:3 * D] = Wi2.T
    bias_all = np.empty(NOUT, np.float32)
    bias_all[0:D] = fuse_b
    bias_all[D:2 * D] = bias_t
    bias_all[2 * D:3 * D] = bias_i
    return W_all, bias_all


def kernel(text_feat, image_feat, fuse_w, fuse_b,
           t_in_w, t_in_b, t_out_w, t_out_b,
           i_in_w, i_in_b, i_out_w, i_out_b):
    global _COMPILED, LAST_RESULTS
    text_feat = np.asarray(text_feat, np.float32)
    image_feat = np.asarray(image_feat, np.float32)
    args = [np.asarray(a, np.float32) for a in
            (fuse_w, fuse_b, t_in_w, t_in_b, t_out_w, t_out_b,
             i_in_w, i_in_b, i_out_w, i_out_b)]
    W_all, bias_all = _fold_params(*args)

    S = text_feat + image_feat                        # (B, D)
    # Per-core pre-tiled S^T: st[m,k,p,q] = S[core*BS + m*128 + q, k*128 + p]
    in_maps = []
    w_arr = np.ascontiguousarray(W_all.reshape(KT, PT, NOUT).astype(NP_DT))
    bi_arr = np.ascontiguousarray(
        np.broadcast_to(bias_all.astype(NP_DT), (PT, NOUT)))
    for c in range(NCORES):
        Sc = S[c * BS:(c + 1) * BS]                   # (BS, D)
        stc = np.ascontiguousarray(
            Sc.reshape(MT, PT, KT, PT).transpose(0, 2, 3, 1).astype(NP_DT))
        in_maps.append({"st": stc, "w": w_arr, "bi": bi_arr})

    if _COMPILED is None:
        _COMPILED = _build()

    LAST_RESULTS = run_bass_kernel_spmd(
        _COMPILED, in_maps, core_ids=list(range(NCORES)))
    outs = np.concatenate(
        [r["out"].astype(np.float32) for r in LAST_RESULTS.results], axis=0)

    fused = outs[:, 0:D]
    out_text = outs[:, D:2 * D]
    out_image = outs[:, 2 * D:3 * D]
    return (out_text, out_image, fused)



# revision 20
# speedup vs baseline: 1.0422x; 1.0422x over previous
"""Trainium2 Bass kernel for nn_CrossModalAttentionFusion.

Math: the module's two MultiheadAttention blocks run with sequence length 1,
so the softmax over a length-1 axis is identically 1.0 and q/k never affect
the output: each MHA reduces to  out = (fused @ Wv.T + bv) @ Wo.T + bo.
Folding the affine chains (done host-side, on the ~1M-param weights only):

    S        = text + image                      (host, elementwise)
    fused    = S @ fuse_w.T + fuse_b
    out_text = S @ (Wt @ fuse_w).T + (Wt @ fuse_b + t_out_w @ t_bv + t_out_b)
               where Wt = t_out_w @ t_wv
    out_image analogously.

Device work: one [B,1024] @ [1024,3072] matmul (the three projections
concatenated along the output dim), sharded batch-parallel over 8 cores.
"""

import numpy as np

import concourse.bass as bass  # noqa: F401  (registers engine methods)
import concourse.mybir as mybir
import concourse.tile as tile
from concourse import bacc
from concourse.bass_utils import run_bass_kernel_spmd

D = 1024
B = 16384
NCORES = 8
BS = B // NCORES          # 2048 rows per core
PT = 128                  # partition tile
MT = BS // PT             # 16 m-tiles per core
KT = D // PT              # 8 k-tiles (contraction)
NOUT = 3 * D              # fused | out_text | out_image
NFREE = 512               # moving free dim per matmul (one PSUM bank of f32)
NT = NOUT // NFREE        # 6

MM_DT = mybir.dt.float16   # fp16 in / fp32 psum accumulate
NP_DT = np.float16
OUT_DT = mybir.dt.float16  # fp16 out (host upcasts); halves output DMA
ORDER = "kn"               # k outer: stationary S-tile serves 6 matmuls (dedup)

_COMPILED = None
LAST_RESULTS = None       # BassKernelResults from the most recent run


def _dedup_ldweights(nc):
    """Remove InstLdweights that reload the stationary already in the PE.

    The bass add_instruction path emits an LDWEIGHTS before every MATMUL.
    With a k-outer/n-inner loop the same stationary serves 6 consecutive
    matmuls, so 5 of 6 loads are redundant; on HW each costs ~50ns of PE
    time (serial with the matmul stream, no pull-ahead for full-array
    loads). Only removes an LDW when (a) its weights AP is byte-identical
    to the previous LDW on the same block with no other LDW between, and
    (b) it carries no semaphore waits/updates (Tile's buffer-reuse gating
    rides on the matmuls' own updates, which are untouched).
    """
    removed = 0
    for f in nc.m.functions:
        for b in f.blocks:
            last_sig = None
            to_remove = []
            for ins in b.instructions:
                tn = type(ins).__name__
                if tn == "InstLdweights":
                    ap = ins.ins[0]
                    sig = (ap.memref, ap.offset, str(ap.ap), str(ap.dtype),
                           getattr(ins, "perf_mode", None),
                           getattr(ins, "is_transpose", None))
                    si = ins.sync_info
                    clean = si is None or (not si.on_wait and not si.on_update)
                    if sig == last_sig and clean:
                        to_remove.append(ins)
                    else:
                        last_sig = sig
                elif tn == "InstMatmult":
                    pass  # keeps the loaded stationary
            for ins in to_remove:
                b.instructions.remove(ins)
                removed += 1
    return removed


def _build(repeat=1, timing=False, order=ORDER, mm_dt=None, micro=None,
           nfree=NFREE, mtiles=MT, skip_w=False, tweak=True, dedup=1,
           hostbias=1):
    """Build the per-core NEFF.

    timing=True swaps the big external tensors for Internal DRAM (no host
    transfer) and adds a `repeat` loop over the whole batch so device time
    can be extracted by wall-clock differencing between two repeat counts.

    order: "nk" = n outer / k inner (sequential psum groups);
           "kn" = k outer / n inner (stationary reused for NT consecutive mms).
    """
    MM_DT = mm_dt if mm_dt is not None else globals()["MM_DT"]
    NT = NOUT // nfree
    psum_bufs = min(8, (8 * 512) // nfree)
    nc = bacc.Bacc("TRN2", target_bir_lowering=False, debug=False,
                   num_devices=NCORES)
    if timing:
        st = nc.dram_tensor("st", [KT, PT, PT], MM_DT, kind="ExternalInput")
        w = nc.dram_tensor("w", [KT, PT, NOUT], MM_DT)
        bi = (None if hostbias else
              nc.dram_tensor("bi", [PT, NOUT], MM_DT))
        out = nc.dram_tensor("out", [BS, NOUT], OUT_DT)
        tok = nc.dram_tensor("tok", [1, 1], mybir.dt.float32,
                             kind="ExternalOutput")
    else:
        st = nc.dram_tensor("st", [MT, KT, PT, PT], MM_DT, kind="ExternalInput")
        w = nc.dram_tensor("w", [KT, PT, NOUT], MM_DT, kind="ExternalInput")
        bi = (None if hostbias else
              nc.dram_tensor("bi", [PT, NOUT], MM_DT, kind="ExternalInput"))
        out = nc.dram_tensor("out", [BS, NOUT], OUT_DT,
                             kind="ExternalOutput")
        tok = None

    with tile.TileContext(nc) as tc:
        with (
            tc.tile_pool(name="wpool", bufs=1) as wpool,
            tc.tile_pool(name="spool", bufs=32 if tweak == 2 else 24) as spool,
            tc.tile_pool(name="opool", bufs=4 if tweak else 3) as opool,
            tc.tile_pool(name="ppool", bufs=psum_bufs, space="PSUM") as ppool,
        ):
            # W/bias (one-time) and output stores ride the ACT HWDGE queue;
            # S loads get the SP queue to themselves so the PE's critical
            # path never waits behind bulk traffic.
            wts = []
            wt_pending = []
            bt = None
            if not skip_w:
                # Even-k W tiles stream on the ACT queue immediately; odd-k
                # tiles ride the SP queue but must be issued AFTER m0's S
                # tiles (deferred into body) so the PE's first stationaries
                # aren't queued behind bulk weights. Interleaved arrival
                # keeps every wt_k ahead of the PE's k-outer consumption.
                for k in range(KT):
                    wt = wpool.tile([PT, NOUT], MM_DT, name=f"wt{k}",
                                    tag=f"wt{k}")
                    if k % 2 == 0:
                        nc.scalar.dma_start(wt[:], w[k])
                    else:
                        wt_pending.append((wt, k))
                    wts.append(wt)
                if not hostbias:
                    bt = wpool.tile([PT, NOUT], MM_DT, name="bt",
                                    tag="bt")

            fixed_sts = None
            if micro in ("mm", "mmdve"):
                fixed_sts = []
                for k in range(KT):
                    s_t = wpool.tile([PT, PT], MM_DT, name=f"sf{k}",
                                     tag=f"sf{k}")
                    nc.sync.dma_start(s_t[:], st[k] if timing else st[0, k])
                    fixed_sts.append(s_t)

            bias_loaded = [False]

            def body():
                for m in range(mtiles):
                    if fixed_sts is not None:
                        sts = fixed_sts
                    else:
                        sts = []
                        for k in range(KT):
                            s_t = spool.tile([PT, PT], MM_DT,
                                             name=f"s{m}_{k}", tag="s")
                            nc.sync.dma_start(s_t[:],
                                              st[k] if timing else st[m, k])
                            sts.append(s_t)
                    if not skip_w and not bias_loaded[0]:
                        # SP queue order after m0's S tiles: odd-k W tiles
                        # first (needed by ~4-14us), then bias if on-device
                        # (first drain needs it at ~15us).
                        for wt, k in wt_pending:
                            nc.sync.dma_start(wt[:], w[k])
                        if bt is not None:
                            nc.sync.dma_start(bt[:], bi[:])
                        bias_loaded[0] = True
                    ot = opool.tile([PT, NOUT], OUT_DT,
                                    name=f"o{m}", tag="o")
                    pts = [ppool.tile([PT, nfree], mybir.dt.float32,
                                      name=f"p{m}_{n}", tag="p")
                           for n in range(NT)]
                    m_order = order
                    if order == "auto":
                        m_order = "kn" if m == 0 else "nk"
                    if m_order == "nk":
                        for n in range(NT):
                            nsl = slice(n * nfree, (n + 1) * nfree)
                            for k in range(KT):
                                nc.tensor.matmul(
                                    pts[n][:], sts[k][:], wts[k][:, nsl],
                                    start=(k == 0), stop=(k == KT - 1),
                                )
                            if tweak and micro != "mm":
                                # drain each group as soon as it closes
                                if bt is None:
                                    nc.vector.tensor_copy(ot[:, nsl],
                                                          pts[n][:])
                                else:
                                    nc.vector.tensor_add(ot[:, nsl], pts[n][:],
                                                         bt[:, nsl])
                            if tweak and micro is None:
                                if tweak == 2:
                                    chunks = {1: slice(0, 1024),
                                              3: slice(1024, 2048),
                                              5: slice(2048, 3072)}
                                else:
                                    chunks = {NT // 2 - 1: slice(0, NOUT // 2),
                                              NT - 1: slice(NOUT // 2, NOUT)}
                                if n in chunks:
                                    half = chunks[n]
                                    nc.scalar.dma_start(
                                        out[m * PT:(m + 1) * PT, half],
                                        ot[:, half])
                    else:  # "kn"
                        for k in range(KT):
                            for n in range(NT):
                                nsl = slice(n * nfree, (n + 1) * nfree)
                                nc.tensor.matmul(
                                    pts[n][:], sts[k][:], wts[k][:, nsl],
                                    start=(k == 0), stop=(k == KT - 1),
                                )
                    drained = tweak and m_order == "nk"
                    if micro != "mm" and not drained:
                        # Evacuate PSUM: bias rides on the host, so drains
                        # are pure copies split across DVE and ACT (they may
                        # touch PSUM concurrently on different banks).
                        for n in range(NT):
                            nsl = slice(n * nfree, (n + 1) * nfree)
                            if bt is not None:
                                nc.vector.tensor_add(ot[:, nsl], pts[n][:],
                                                     bt[:, nsl])
                            elif n % 2 == 0:
                                nc.vector.tensor_copy(ot[:, nsl], pts[n][:])
                            else:
                                nc.scalar.copy(ot[:, nsl], pts[n][:])
                    if micro is None and not drained:
                        nc.scalar.dma_start(out[m * PT:(m + 1) * PT, :], ot[:])

            if repeat > 1:
                with tc.For_i(0, repeat, 1,
                              hint_engines=(mybir.EngineType.PE,)):
                    body()
            else:
                body()
            if tok is not None:
                tk = wpool.tile([1, 1], mybir.dt.float32, name="tk", tag="tk")
                nc.gpsimd.memset(tk[:], 1.0)
                nc.sync.dma_start(tok[:], tk[:])

    if dedup:
        _dedup_ldweights(nc)
    nc.compile()
    return nc


def _fold_params(fuse_w, fuse_b, t_in_w, t_in_b, t_out_w, t_out_b,
                 i_in_w, i_in_b, i_out_w, i_out_b):
    """Host-side weight folding (float64). Returns W_all [D, 3D], bias_all [3D]."""
    f8 = np.float64
    fuse_w8, fuse_b8 = fuse_w.astype(f8), fuse_b.astype(f8)

    def fold(in_w, in_b, out_w, out_b):
        wv = in_w[2 * D:3 * D].astype(f8)
        bv = in_b[2 * D:3 * D].astype(f8)
        Wm = out_w.astype(f8) @ wv                    # fused -> out
        bm = out_w.astype(f8) @ bv + out_b.astype(f8)
        W2 = Wm @ fuse_w8                             # S -> out
        b2 = Wm @ fuse_b8 + bm
        return W2, b2

    Wt2, bias_t = fold(t_in_w, t_in_b, t_out_w, t_out_b)
    Wi2, bias_i = fold(i_in_w, i_in_b, i_out_w, i_out_b)

    W_all = np.empty((D, NOUT), np.float32)
    W_all[:, 0:D] = fuse_w8.T
    W_all[:, D:2 * D] = Wt2.T
    W_all[:, 2 * D:3 * D] = Wi2.T
    bias_all = np.empty(NOUT, np.float32)
    bias_all[0:D] = fuse_b
    bias_all[D:2 * D] = bias_t
    bias_all[2 * D:3 * D] = bias_i
    return W_all, bias_all


def kernel(text_feat, image_feat, fuse_w, fuse_b,
           t_in_w, t_in_b, t_out_w, t_out_b,
           i_in_w, i_in_b, i_out_w, i_out_b):
    global _COMPILED, LAST_RESULTS
    text_feat = np.asarray(text_feat, np.float32)
    image_feat = np.asarray(image_feat, np.float32)
    args = [np.asarray(a, np.float32) for a in
            (fuse_w, fuse_b, t_in_w, t_in_b, t_out_w, t_out_b,
             i_in_w, i_in_b, i_out_w, i_out_b)]
    W_all, bias_all = _fold_params(*args)

    S = text_feat + image_feat                        # (B, D)
    # Per-core pre-tiled S^T: st[m,k,p,q] = S[core*BS + m*128 + q, k*128 + p]
    in_maps = []
    w_arr = np.ascontiguousarray(W_all.reshape(KT, PT, NOUT).astype(NP_DT))
    for c in range(NCORES):
        Sc = S[c * BS:(c + 1) * BS]                   # (BS, D)
        stc = np.ascontiguousarray(
            Sc.reshape(MT, PT, KT, PT).transpose(0, 2, 3, 1).astype(NP_DT))
        in_maps.append({"st": stc, "w": w_arr})

    if _COMPILED is None:
        _COMPILED = _build()

    LAST_RESULTS = run_bass_kernel_spmd(
        _COMPILED, in_maps, core_ids=list(range(NCORES)))
    outs = np.concatenate(
        [r["out"].astype(np.float32) for r in LAST_RESULTS.results], axis=0)
    outs += bias_all[None, :]

    fused = outs[:, 0:D]
    out_text = outs[:, D:2 * D]
    out_image = outs[:, 2 * D:3 * D]
    return (out_text, out_image, fused)



# revision 27
# speedup vs baseline: 1.1147x; 1.0695x over previous
"""Trainium2 Bass kernel for nn_CrossModalAttentionFusion.

Math: the module's two MultiheadAttention blocks run with sequence length 1,
so the softmax over a length-1 axis is identically 1.0 and q/k never affect
the output: each MHA reduces to  out = (fused @ Wv.T + bv) @ Wo.T + bo.
Folding the affine chains (done host-side, on the ~1M-param weights only):

    S        = text + image                      (host, elementwise)
    fused    = S @ fuse_w.T + fuse_b
    out_text = S @ (Wt @ fuse_w).T + (Wt @ fuse_b + t_out_w @ t_bv + t_out_b)
               where Wt = t_out_w @ t_wv
    out_image analogously.

Device work: one [B,1024] @ [1024,3072] matmul (the three projections
concatenated along the output dim), sharded batch-parallel over 8 cores.
"""

import numpy as np

import concourse.bass as bass  # noqa: F401  (registers engine methods)
import concourse.mybir as mybir
import concourse.tile as tile
from concourse import bacc
from concourse.bass_utils import run_bass_kernel_spmd

D = 1024
B = 16384
NCORES = 8
BS = B // NCORES          # 2048 rows per core
PT = 128                  # partition tile
MT = BS // PT             # 16 m-tiles per core
KT = D // PT              # 8 k-tiles (contraction)
NOUT = 3 * D              # fused | out_text | out_image
NFREE = 512               # moving free dim per matmul (one PSUM bank of f32)
NT = NOUT // NFREE        # 6

MM_DT = mybir.dt.float16   # fp16 in / fp32 psum accumulate
NP_DT = np.float16
OUT_DT = mybir.dt.float16  # fp16 out (host upcasts); halves output DMA
ORDER = "kn"               # k outer: stationary S-tile serves 6 matmuls (dedup)

_COMPILED = None
LAST_RESULTS = None       # BassKernelResults from the most recent run


def _dedup_ldweights(nc):
    """Remove InstLdweights that reload the stationary already in the PE.

    The bass add_instruction path emits an LDWEIGHTS before every MATMUL.
    With a k-outer/n-inner loop the same stationary serves 6 consecutive
    matmuls, so 5 of 6 loads are redundant; on HW each costs ~50ns of PE
    time (serial with the matmul stream, no pull-ahead for full-array
    loads). Only removes an LDW when (a) its weights AP is byte-identical
    to the previous LDW on the same block with no other LDW between, and
    (b) it carries no semaphore waits/updates (Tile's buffer-reuse gating
    rides on the matmuls' own updates, which are untouched).
    """
    removed = 0
    for f in nc.m.functions:
        for b in f.blocks:
            last_sig = None
            to_remove = []
            for ins in b.instructions:
                tn = type(ins).__name__
                if tn == "InstLdweights":
                    ap = ins.ins[0]
                    sig = (ap.memref, ap.offset, str(ap.ap), str(ap.dtype),
                           getattr(ins, "perf_mode", None),
                           getattr(ins, "is_transpose", None))
                    si = ins.sync_info
                    clean = si is None or (not si.on_wait and not si.on_update)
                    if sig == last_sig and clean:
                        to_remove.append(ins)
                    else:
                        last_sig = sig
                elif tn == "InstMatmult":
                    pass  # keeps the loaded stationary
            for ins in to_remove:
                b.instructions.remove(ins)
                removed += 1
    return removed


def _build(repeat=1, timing=False, order=ORDER, mm_dt=None, micro=None,
           nfree=NFREE, mtiles=MT, skip_w=False, tweak=True, dedup=1,
           hostbias=1):
    """Build the per-core NEFF.

    timing=True swaps the big external tensors for Internal DRAM (no host
    transfer) and adds a `repeat` loop over the whole batch so device time
    can be extracted by wall-clock differencing between two repeat counts.

    order: "nk" = n outer / k inner (sequential psum groups);
           "kn" = k outer / n inner (stationary reused for NT consecutive mms).
    """
    MM_DT = mm_dt if mm_dt is not None else globals()["MM_DT"]
    NT = NOUT // nfree
    psum_bufs = min(8, (8 * 512) // nfree)
    nc = bacc.Bacc("TRN2", target_bir_lowering=False, debug=False,
                   num_devices=NCORES)
    if timing:
        st = nc.dram_tensor("st", [KT, PT, PT], MM_DT, kind="ExternalInput")
        w = nc.dram_tensor("w", [KT, PT, NOUT], MM_DT)
        bi = (None if hostbias else
              nc.dram_tensor("bi", [PT, NOUT], MM_DT))
        out = nc.dram_tensor("out", [BS, NOUT], OUT_DT)
        tok = nc.dram_tensor("tok", [1, 1], mybir.dt.float32,
                             kind="ExternalOutput")
    else:
        st = nc.dram_tensor("st", [MT, KT, PT, PT], MM_DT, kind="ExternalInput")
        w = nc.dram_tensor("w", [KT, PT, NOUT], MM_DT, kind="ExternalInput")
        bi = (None if hostbias else
              nc.dram_tensor("bi", [PT, NOUT], MM_DT, kind="ExternalInput"))
        out = nc.dram_tensor("out", [BS, NOUT], OUT_DT,
                             kind="ExternalOutput")
        tok = None

    with tile.TileContext(nc) as tc:
        with (
            tc.tile_pool(name="wpool", bufs=1) as wpool,
            tc.tile_pool(name="spool", bufs=32 if tweak == 2 else 24) as spool,
            tc.tile_pool(name="opool", bufs=4 if tweak else 3) as opool,
            tc.tile_pool(name="ppool", bufs=psum_bufs, space="PSUM") as ppool,
        ):
            # W/bias (one-time) and output stores ride the ACT HWDGE queue;
            # S loads get the SP queue to themselves so the PE's critical
            # path never waits behind bulk traffic.
            wts = []
            wt_pending = []
            bt = None
            WCH = 1024            # W DMA chunk: 3 chunks/k-tile, 256KB each
            if not skip_w:
                # W streams in k-major 1024-col chunks, alternating between
                # the ACT and SP HWDGE queues, so the PE's k-outer
                # consumption (one k-tile per ~1.3us) never outruns arrival
                # and the FIRST matmul only waits for one 256KB chunk, not
                # a whole 768KB k-tile. SP-queue chunks are deferred into
                # the body (after m0's S tiles) so the first stationaries
                # aren't queued behind bulk weights.
                for k in range(KT):
                    wt = wpool.tile([PT, NOUT], MM_DT, name=f"wt{k}",
                                    tag=f"wt{k}")
                    for ci in range(NOUT // WCH):
                        csl = slice(ci * WCH, (ci + 1) * WCH)
                        if (k * (NOUT // WCH) + ci) % 2 == 0:
                            nc.scalar.dma_start(wt[:, csl], w[k][:, csl])
                        else:
                            wt_pending.append((wt, k, csl))
                    wts.append(wt)
                if not hostbias:
                    bt = wpool.tile([PT, NOUT], MM_DT, name="bt",
                                    tag="bt")

            fixed_sts = None
            if micro in ("mm", "mmdve"):
                fixed_sts = []
                for k in range(KT):
                    s_t = wpool.tile([PT, PT], MM_DT, name=f"sf{k}",
                                     tag=f"sf{k}")
                    nc.sync.dma_start(s_t[:], st[k] if timing else st[0, k])
                    fixed_sts.append(s_t)

            bias_loaded = [False]

            def body():
                for m in range(mtiles):
                    if fixed_sts is not None:
                        sts = fixed_sts
                    else:
                        sts = []
                        for k in range(KT):
                            s_t = spool.tile([PT, PT], MM_DT,
                                             name=f"s{m}_{k}", tag="s")
                            nc.sync.dma_start(s_t[:],
                                              st[k] if timing else st[m, k])
                            sts.append(s_t)
                            if not skip_w and not bias_loaded[0] and k == 0:
                                # k0's odd W chunk rides right behind
                                # S[m0,k0] so the PE's first k-group never
                                # waits on the back of the S burst.
                                for wt, wk, csl in wt_pending:
                                    if wk == 0:
                                        nc.sync.dma_start(wt[:, csl],
                                                          w[wk][:, csl])
                    if not skip_w and not bias_loaded[0]:
                        # SP queue after m0's S tiles: remaining odd W
                        # chunks in k-major order, then bias if on-device.
                        for wt, k, csl in wt_pending:
                            if k > 0:
                                nc.sync.dma_start(wt[:, csl], w[k][:, csl])
                        if bt is not None:
                            nc.sync.dma_start(bt[:], bi[:])
                        bias_loaded[0] = True
                    ot = opool.tile([PT, NOUT], OUT_DT,
                                    name=f"o{m}", tag="o")
                    pts = [ppool.tile([PT, nfree], mybir.dt.float32,
                                      name=f"p{m}_{n}", tag="p")
                           for n in range(NT)]
                    m_order = order
                    if order == "auto":
                        m_order = "kn" if m == 0 else "nk"
                    if m_order == "nk":
                        for n in range(NT):
                            nsl = slice(n * nfree, (n + 1) * nfree)
                            for k in range(KT):
                                nc.tensor.matmul(
                                    pts[n][:], sts[k][:], wts[k][:, nsl],
                                    start=(k == 0), stop=(k == KT - 1),
                                )
                            if tweak and micro != "mm":
                                # drain each group as soon as it closes,
                                # alternating DVE/ACT
                                if bt is not None:
                                    nc.vector.tensor_add(ot[:, nsl], pts[n][:],
                                                         bt[:, nsl])
                                elif n % 2 == 0:
                                    nc.vector.tensor_copy(ot[:, nsl],
                                                          pts[n][:])
                                else:
                                    nc.scalar.copy(ot[:, nsl], pts[n][:])
                            if tweak and micro is None:
                                if tweak == 2:
                                    chunks = {1: slice(0, 1024),
                                              3: slice(1024, 2048),
                                              5: slice(2048, 3072)}
                                else:
                                    chunks = {NT // 2 - 1: slice(0, NOUT // 2),
                                              NT - 1: slice(NOUT // 2, NOUT)}
                                if n in chunks:
                                    half = chunks[n]
                                    nc.sync.dma_start(
                                        out[m * PT:(m + 1) * PT, half],
                                        ot[:, half])
                    else:  # "kn"
                        for k in range(KT):
                            for n in range(NT):
                                nsl = slice(n * nfree, (n + 1) * nfree)
                                nc.tensor.matmul(
                                    pts[n][:], sts[k][:], wts[k][:, nsl],
                                    start=(k == 0), stop=(k == KT - 1),
                                )
                    drained = tweak and m_order == "nk"
                    if micro != "mm" and not drained:
                        # Evacuate PSUM: bias rides on the host, so drains
                        # are pure copies split across DVE and ACT (they may
                        # touch PSUM concurrently on different banks). The
                        # out DMA goes in halves so the first half streams
                        # while the second half is still draining — this is
                        # what bounds the kernel tail after the last matmul.
                        last_m = m == mtiles - 1
                        for n in range(NT):
                            nsl = slice(n * nfree, (n + 1) * nfree)
                            if bt is not None:
                                nc.vector.tensor_add(ot[:, nsl], pts[n][:],
                                                     bt[:, nsl])
                            elif n % 2 == (1 if last_m else 0):
                                nc.vector.tensor_copy(ot[:, nsl], pts[n][:])
                            else:
                                nc.scalar.copy(ot[:, nsl], pts[n][:])
                            if micro is None and last_m:
                                # last tile: store per group so the kernel
                                # ends on a 128KB DMA, not a 384KB one.
                                # Out stores issue on the SP queue: HWDGE
                                # issue costs ~0.5us of the issuing engine,
                                # which must not serialize with ACT drains.
                                nc.sync.dma_start(
                                    out[m * PT:(m + 1) * PT, nsl],
                                    ot[:, nsl])
                            elif micro is None and n == NT // 2 - 1:
                                half = slice(0, (n + 1) * nfree)
                                nc.sync.dma_start(
                                    out[m * PT:(m + 1) * PT, half],
                                    ot[:, half])
                        if micro is None and not last_m:
                            half = slice(NOUT // 2, NOUT)
                            nc.sync.dma_start(
                                out[m * PT:(m + 1) * PT, half], ot[:, half])

            if repeat > 1:
                with tc.For_i(0, repeat, 1,
                              hint_engines=(mybir.EngineType.PE,)):
                    body()
            else:
                body()
            if tok is not None:
                tk = wpool.tile([1, 1], mybir.dt.float32, name="tk", tag="tk")
                nc.gpsimd.memset(tk[:], 1.0)
                nc.sync.dma_start(tok[:], tk[:])

    if dedup:
        _dedup_ldweights(nc)
    nc.compile()
    return nc


def _fold_params(fuse_w, fuse_b, t_in_w, t_in_b, t_out_w, t_out_b,
                 i_in_w, i_in_b, i_out_w, i_out_b):
    """Host-side weight folding (float64). Returns W_all [D, 3D], bias_all [3D]."""
    f8 = np.float64
    fuse_w8, fuse_b8 = fuse_w.astype(f8), fuse_b.astype(f8)

    def fold(in_w, in_b, out_w, out_b):
        wv = in_w[2 * D:3 * D].astype(f8)
        bv = in_b[2 * D:3 * D].astype(f8)
        Wm = out_w.astype(f8) @ wv                    # fused -> out
        bm = out_w.astype(f8) @ bv + out_b.astype(f8)
        W2 = Wm @ fuse_w8                             # S -> out
        b2 = Wm @ fuse_b8 + bm
        return W2, b2

    Wt2, bias_t = fold(t_in_w, t_in_b, t_out_w, t_out_b)
    Wi2, bias_i = fold(i_in_w, i_in_b, i_out_w, i_out_b)

    W_all = np.empty((D, NOUT), np.float32)
    W_all[:, 0:D] = fuse_w8.T
    W_all[:, D:2 * D] = Wt2.T
    W_all[:, 2 * D:3 * D] = Wi2.T
    bias_all = np.empty(NOUT, np.float32)
    bias_all[0:D] = fuse_b
    bias_all[D:2 * D] = bias_t
    bias_all[2 * D:3 * D] = bias_i
    return W_all, bias_all


def kernel(text_feat, image_feat, fuse_w, fuse_b,
           t_in_w, t_in_b, t_out_w, t_out_b,
           i_in_w, i_in_b, i_out_w, i_out_b):
    global _COMPILED, LAST_RESULTS
    text_feat = np.asarray(text_feat, np.float32)
    image_feat = np.asarray(image_feat, np.float32)
    args = [np.asarray(a, np.float32) for a in
            (fuse_w, fuse_b, t_in_w, t_in_b, t_out_w, t_out_b,
             i_in_w, i_in_b, i_out_w, i_out_b)]
    W_all, bias_all = _fold_params(*args)

    S = text_feat + image_feat                        # (B, D)
    # Per-core pre-tiled S^T: st[m,k,p,q] = S[core*BS + m*128 + q, k*128 + p]
    in_maps = []
    w_arr = np.ascontiguousarray(W_all.reshape(KT, PT, NOUT).astype(NP_DT))
    for c in range(NCORES):
        Sc = S[c * BS:(c + 1) * BS]                   # (BS, D)
        stc = np.ascontiguousarray(
            Sc.reshape(MT, PT, KT, PT).transpose(0, 2, 3, 1).astype(NP_DT))
        in_maps.append({"st": stc, "w": w_arr})

    if _COMPILED is None:
        _COMPILED = _build()

    LAST_RESULTS = run_bass_kernel_spmd(
        _COMPILED, in_maps, core_ids=list(range(NCORES)))
    outs = np.concatenate(
        [r["out"].astype(np.float32) for r in LAST_RESULTS.results], axis=0)
    outs += bias_all[None, :]

    fused = outs[:, 0:D]
    out_text = outs[:, D:2 * D]
    out_image = outs[:, 2 * D:3 * D]
    return (out_text, out_image, fused)

